# revision 42
# baseline (speedup 1.0000x reference)
"""3-layer GAT (gnn_message_passing) on 8 Trainium2 NeuronCores.

Strategy: nodes sharded by dst octant.  Per layer:
  - node-parallel matmul phase computes Z=[h|es] rows for the core's 6250
    nodes plus a per-tile ed table kept in SBUF
  - chunked AllGather of the Z table (bf16 rows, 1280B) overlapped with the
    matmul phase via a chunk-major permuted table row layout
  - dst-pair aggregation: dma_gather edge messages by src id into a merged
    A|B buffer, ed broadcast to slots via mskT@ed tensor matmuls
    (host-precomputed transposed selection masks, streamed), logits
    exp(lrelu(es+ed)) on the scalar engine, message scaling on vector,
    host-precomputed selection-mask matmuls accumulate numerator+denominator
    in PSUM across all chunk segments, epilogue normalizes + bias (+relu)
    and transposes into the next layer's matmul input.
Edge structure (sorting by (pair, A/B group, tile, dst), chunk-aligned
A/B regions padded to the cross-core max, selection masks and their
transposes) is precomputed on host; dummy table rows carry es=-1e9 so
padded slots get exp(-inf)=0 weight.
"""
import os
import sys

sys.path.insert(0, "/opt/trn_rl_repo")

import numpy as np
import ml_dtypes

BF16NP = ml_dtypes.bfloat16
NEG_BIG = -1.0e9

MAX_WAITS = 1


def _split_multiwait(nc):
    """walrus in this env rejects >1 sync-wait per instruction: split excess
    waits onto same-engine NoOps."""
    import concourse.mybir as mybir
    for _name, bbb in nc.bb_map.items():
        il = bbb.bb.instructions
        new = []
        changed = False
        for inst in il:
            si = getattr(inst, "sync_info", None)
            ow = list(si.on_wait) if si is not None and si.on_wait else []
            if len(ow) > MAX_WAITS:
                excess, keep = ow[:-MAX_WAITS], ow[-MAX_WAITS:]
                for j, w in enumerate(excess):
                    new.append(mybir.InstNoOp(
                        name=f"{inst.name}_sw{j}",
                        engine=inst.engine,
                        bass_nofuse=True,
                        sync_info=mybir.SyncInfo(on_wait=[w], on_update=[]),
                    ))
                inst.sync_info = mybir.SyncInfo(
                    on_wait=keep, on_update=list(si.on_update))
                changed = True
            new.append(inst)
        if changed:
            bbb.bb.instructions = new


# --------------------------------------------------------------------------
# host-side edge preprocessing
# --------------------------------------------------------------------------
def wrap16(vals):
    """dma_gather index layout: element i -> [i%16, i//16]; [128, n//16]."""
    v = np.asarray(vals, dtype=np.int64)
    num = len(v)
    assert num % 16 == 0
    a = np.zeros((16, num // 16), dtype=np.int16)
    i = np.arange(num)
    a[i % 16, i // 16] = v.astype(np.int16)
    return np.tile(a, (8, 1))


def build_perm(N, P, R, tile_ranges):
    """Chunk-major table row permutation.  Table rows (1-based, dummies at 0
    and N+1): row = 1 + off_k + oct*rows_k + (loc - r0_k)."""
    TPC = (R + 127) // 128
    perm = np.zeros(N, np.int64)
    off = 0
    offs = []
    rows_ks = []
    for (t0, t1) in tile_ranges:
        r0 = t0 * 128
        r1 = min(t1 * 128, R)
        rows_k = r1 - r0
        offs.append(off)
        rows_ks.append(rows_k)
        off += P * rows_k
    node = np.arange(N)
    oct_ = node // R
    loc = node % R
    for k, (t0, t1) in enumerate(tile_ranges):
        r0 = t0 * 128
        r1 = min(t1 * 128, R)
        sel = (loc >= r0) & (loc < r1)
        perm[sel] = 1 + offs[k] + oct_[sel] * rows_ks[k] + (loc[sel] - r0)
    return perm, offs, rows_ks


def build_edges2(edge_index, N, P, R, TPC, ATH, perm):
    """Per-core gather index streams + per-pair metadata + selection masks.

    Per pair (2 dst tiles): A region (perm row < ATH) then B region, each
    chunk-aligned (128 slots), sizes = max over cores.  Masks are emitted per
    host-shared (chunk, tile) descriptor; per-core mask data zero-filled
    where a core has no slots.
    """
    # self-loops are handled locally in the epilogue, not gathered
    src = edge_index[0].astype(np.int64)
    dst = edge_index[1].astype(np.int64)
    owner = dst // R
    NPAIR = (TPC + 1) // 2

    # per (core, pair): sorted slot lists
    core_pair = [[None] * NPAIR for _ in range(P)]
    nA = np.zeros((P, NPAIR), np.int64)
    nB = np.zeros((P, NPAIR), np.int64)
    for p in range(P):
        sel = np.nonzero(owner == p)[0]
        d = dst[sel] - p * R
        row = perm[src[sel]]
        grp = (row >= ATH).astype(np.int64)
        t = d // 128
        pair = t // 2
        order = np.lexsort((d, t, grp, pair))
        d = d[order]; row = row[order]; grp = grp[order]
        t = t[order]; pair = pair[order]
        bounds = np.searchsorted(pair, np.arange(NPAIR + 1))
        for g in range(NPAIR):
            s0, s1 = bounds[g], bounds[g + 1]
            gg = grp[s0:s1]
            na = int(np.searchsorted(gg, 1))
            core_pair[p][g] = (d[s0:s1], row[s0:s1], t[s0:s1], na)
            nA[p, g] = na
            nB[p, g] = (s1 - s0) - na

    # shared per-pair chunk counts
    nA128 = (nA.max(axis=0) + 127) // 128
    nB128 = (nB.max(axis=0) + 127) // 128
    nA128 = np.maximum(nA128, 1)
    nB128 = np.maximum(nB128, 1)

    pair_meta = []
    CA = CB = 0
    Mtot = 0
    for g in range(NPAIR):
        tiles = [2 * g] + ([2 * g + 1] if 2 * g + 1 < TPC else [])
        na_c = int(nA128[g])
        nb_c = int(nB128[g])
        nck = na_c + nb_c
        # union descriptors (j, ti): chunk j intersects tile ti on any core
        desc = []
        seen = set()
        for p in range(P):
            d, row, t, na = core_pair[p][g]
            for (lo, n, base_ck) in ((0, na, 0), (na, len(d) - na, na_c)):
                tt = t[lo:lo + n]
                for ti in range(len(tiles)):
                    idx = np.nonzero(tt == tiles[ti])[0]
                    if len(idx) == 0:
                        continue
                    c0 = base_ck + idx[0] // 128
                    c1 = base_ck + idx[-1] // 128
                    for j in range(c0, c1 + 1):
                        seen.add((j, ti))
        desc = sorted(seen)
        pair_meta.append(dict(tiles=tiles, na_c=na_c, nb_c=nb_c, nck=nck,
                              desc=desc, m0=Mtot, a0=CA, b0=CB))
        CA += na_c
        CB += nb_c
        Mtot += len(desc)

    out = []
    for p in range(P):
        idxA_parts = []
        idxB_parts = []
        msk = np.zeros((Mtot * 128, 128), np.float32)   # [slotblock, d] rows
        for g in range(NPAIR):
            pm = pair_meta[g]
            d, row, t, na = core_pair[p][g]
            na_c, nb_c = pm["na_c"], pm["nb_c"]
            va = np.zeros(na_c * 128, np.int64)          # pad -> dummy row 0
            va[:na] = row[:na]
            vb = np.full(nb_c * 128, N + 1 - ATH, np.int64)
            vb[:len(d) - na] = row[na:] - ATH
            idxA_parts.append(va)
            idxB_parts.append(vb)
            # slot -> (chunk, within) ; chunk = A chunks then B chunks
            slot_ck = np.concatenate([
                np.arange(na) // 128,
                na_c + np.arange(len(d) - na) // 128])
            slot_in = np.concatenate([
                np.arange(na) % 128, np.arange(len(d) - na) % 128])
            dloc = d - (t * 128)
            for m, (j, ti) in enumerate(pm["desc"]):
                sel = np.nonzero((slot_ck == j) & (t == pm["tiles"][ti]))[0]
                if len(sel) == 0:
                    continue
                blk = (pm["m0"] + m) * 128
                msk[blk + slot_in[sel], dloc[sel]] = 1.0
        idxA = wrap16(np.concatenate(idxA_parts))
        idxB = wrap16(np.concatenate(idxB_parts))
        # msk view [Mtot, 128slot, 128d] -> device msk [128slot, Mtot*128d]
        m3 = msk.reshape(Mtot, 128, 128)
        mskd = np.ascontiguousarray(
            m3.transpose(1, 0, 2).reshape(128, Mtot * 128)).astype(BF16NP)
        mskT = np.ascontiguousarray(
            m3.transpose(2, 0, 1).reshape(128, Mtot * 128)).astype(BF16NP)
        out.append({"idxA": idxA, "idxB": idxB, "msk": mskd, "mskT": mskT})
    return out, pair_meta, CA, CB, Mtot


# --------------------------------------------------------------------------
# device program
# --------------------------------------------------------------------------
def build_program(cfg, pair_meta, CA, CB, Mtot, ag_chunks):
    import concourse.bass as bass
    import concourse.mybir as mybir
    import concourse.tile as tile
    from concourse.library_config import mlp
    from concourse.masks import make_identity
    from concourse.tile_rust import add_dep_helper

    def _mi(x):
        return getattr(x, "ins", x)

    def dep(a, b, why):
        add_dep_helper(_mi(a), _mi(b), reason=why)

    F32 = mybir.dt.float32
    BF16 = mybir.dt.bfloat16
    I16 = mybir.dt.int16

    N, P, R, TPC = cfg["N"], cfg["P"], cfg["R"], cfg["TPC"]
    F_IN, HID, HEADS, OUT = cfg["F_IN"], cfg["HID"], cfg["HEADS"], cfg["OUT"]
    ATH = cfg["ATH"]
    HC = HID * HEADS
    IN2 = HC + F_IN
    TROW = cfg["TROW"]
    TROW3 = cfg["TROW3"]
    NRT = N + 2
    NTILE = TPC * 128
    NCKMAX = max(pm["nck"] for pm in pair_meta)
    MMAX = max(len(pm["desc"]) for pm in pair_meta)
    NEG_SLOPE = cfg["NEG_SLOPE"]

    nc = bass.Bass()

    ps = {}

    def par(name, shape, dt):
        ps[name] = nc.declare_dram_parameter(name, list(shape), dt,
                                             isOutput=False)
        return ps[name]

    xT = par("xT", [TPC * (F_IN // 128) * 128, 128], BF16)
    Wm1 = par("Wm1", [F_IN, HC], BF16)
    Wa1 = par("Wa1", [F_IN, 2 * HEADS], BF16)
    Wm2 = par("Wm2", [IN2, HC], BF16)
    Wa2 = par("Wa2", [IN2, 2 * HEADS], BF16)
    Wm3 = par("Wm3", [IN2, OUT], BF16)
    Wa3 = par("Wa3", [IN2, 2], BF16)
    b1 = par("b1", [128, HC], F32)
    b2 = par("b2", [128, HC], F32)
    b3 = par("b3", [128, OUT], F32)
    idxA_p = par("idxA", [128, CA * 8], I16)
    idxB_p = par("idxB", [128, CB * 8], I16)
    msk_p = par("msk", [128, Mtot * 128], BF16)
    mskT_p = par("mskT", [128, Mtot * 128], BF16)
    dum640 = par("dum640", [1, TROW], BF16)
    dum128 = par("dum128", [1, TROW3], BF16)
    out_ext = nc.declare_dram_parameter("out", [R, OUT], F32, isOutput=True)

    DBGL = int(os.environ.get("GNN_DEBUG", "0"))
    DBG = DBGL > 0
    dbg = {}
    if DBG:
        NPAIR = len(pair_meta)
        for nm, shape, dt in (
                ("dbg_T1", [512, TROW], BF16),
                ("dbg_buf", [128, 8 * TROW], BF16),
                ("dbg_mk", [128, 8 * 128], BF16),
                ("dbg_mkT", [128, 8 * 128], BF16),
                ("dbg_et", [128, 32 * 8], F32),
                ("dbg_ex", [128, 32 * 8], BF16),
                ("dbg_eds", [128, 32 * 8], F32),
                ("dbg_denall", [NPAIR * 128, 16], F32),
                ("dbg_o1all", [NPAIR * 128, 512], F32),
                ("dbg_ed", [128, 49 * 8], BF16),
                ("dbg_outT", [128, 4 * 6272], BF16),
        ):
            dbg[nm] = nc.declare_dram_parameter(nm, shape, dt, isOutput=True)

    T1 = nc.dram_tensor("T1", [NRT, TROW], BF16, addr_space="Shared")
    T2 = nc.dram_tensor("T2", [NRT, TROW], BF16, addr_space="Shared")
    T3 = nc.dram_tensor("T3", [NRT, TROW3], BF16, addr_space="Shared")
    T1sh = nc.dram_tensor("T1sh", [R, TROW], BF16)
    T2sh = nc.dram_tensor("T2sh", [R, TROW], BF16)
    T3sh = nc.dram_tensor("T3sh", [R, TROW3], BF16)

    nc.gpsimd.load_library(mlp)

    from contextlib import ExitStack
    _regstack = ExitStack()
    _regcache = {}

    def numreg(v):
        if v not in _regcache:
            r = _regstack.enter_context(nc.gpsimd.register(f"nidx{v}"))
            nc.gpsimd.reg_mov(r, v)
            _regcache[v] = r
        return _regcache[v]

    with tile.TileContext(nc) as tc:
        with ExitStack() as _pools:
            ep_ = _pools.enter_context
            constp = ep_(tc.tile_pool(name="const", bufs=1))
            wp = ep_(tc.tile_pool(name="w", bufs=1))
            xtp = ep_(tc.tile_pool(name="xt", bufs=8))
            outTp = ep_(tc.tile_pool(name="outT", bufs=1))
            edsp = ep_(tc.tile_pool(name="eds", bufs=1))
            mmzp = ep_(tc.tile_pool(name="mmz", bufs=2))
            gabp = ep_(tc.tile_pool(name="gab", bufs=2))
            mkp = ep_(tc.tile_pool(name="mk", bufs=2))
            mkTp = ep_(tc.tile_pool(name="mkT", bufs=2))
            lgtp = ep_(tc.tile_pool(name="lgt", bufs=2))
            mpp = ep_(tc.tile_pool(name="mp", bufs=2))
            epp = ep_(tc.tile_pool(name="ep", bufs=2))
            psAp = ep_(tc.tile_pool(name="psA", bufs=2, space="PSUM"))
            psDenp = ep_(tc.tile_pool(name="psDen", bufs=1, space="PSUM"))
            psEp = ep_(tc.tile_pool(name="psE", bufs=1, space="PSUM"))
            psTp = ep_(tc.tile_pool(name="psT", bufs=1, space="PSUM"))
            pmmp = ep_(tc.tile_pool(name="pmm", bufs=2, space="PSUM"))
            pamp = ep_(tc.tile_pool(name="pam", bufs=1, space="PSUM"))
            sxp = ep_(tc.tile_pool(name="sx", bufs=1))
            ztp = ep_(tc.tile_pool(name="zt", bufs=2))
            # ---------- constants / resident data
            ident = constp.tile([128, 128], BF16, tag="ident")
            make_identity(nc, ident[:])

            idxA_sb = constp.tile([128, CA * 8], I16, tag="idxA")
            nc.sync.dma_start(out=idxA_sb[:], in_=idxA_p[:])
            idxB_sb = constp.tile([128, CB * 8], I16, tag="idxB")
            nc.sync.dma_start(out=idxB_sb[:], in_=idxB_p[:])

            bias_sb = {}
            for nm, p_, w_ in (("b1", b1, HC), ("b2", b2, HC), ("b3", b3, OUT)):
                bias_sb[nm] = constp.tile([128, w_], F32, tag=nm, name=nm)
                nc.sync.dma_start(out=bias_sb[nm][:], in_=p_[:])

            dummy_w = {}
            for T_, dum in ((T1, dum640), (T2, dum640), (T3, dum128)):
                i1 = nc.sync.dma_start(out=T_[0:1, :], in_=dum[:])
                i2 = nc.sync.dma_start(out=T_[N + 1:N + 2, :], in_=dum[:])
                dummy_w[id(T_)] = [i1, i2]

            def load_w(p_, rows, cols, tag):
                nchunks = (rows + 127) // 128
                tl = wp.tile([128, nchunks * cols], BF16, tag=tag)
                for fc in range(nchunks):
                    r0 = fc * 128
                    vr = min(128, rows - r0)
                    nc.sync.dma_start(out=tl[:vr, fc * cols:(fc + 1) * cols],
                                      in_=p_[r0:r0 + vr, :])
                return tl

            Wm1_sb = load_w(Wm1, F_IN, HC, "Wm1")
            Wa1_sb = load_w(Wa1, F_IN, 2 * HEADS, "Wa1")
            Wm2_sb = load_w(Wm2, IN2, HC, "Wm2")
            Wa2_sb = load_w(Wa2, IN2, 2 * HEADS, "Wa2")
            Wm3_sb = load_w(Wm3, IN2, OUT, "Wm3")
            Wa3_sb = load_w(Wa3, IN2, 2, "Wa3")

            outT_sb = outTp.tile([128, (HC // 128) * NTILE], BF16, tag="outT")
            # per-layer ed tables (bf16), written by matmul phase
            ed_sb = {
                1: edsp.tile([128, TPC * HEADS], BF16, tag="ed1", name="ed1"),
                2: edsp.tile([128, TPC * HEADS], BF16, tag="ed2", name="ed2"),
                3: edsp.tile([128, TPC * 1], BF16, tag="ed3", name="ed3"),
            }
            es_sb = {
                1: edsp.tile([128, TPC * HEADS], F32, tag="es1", name="es1"),
                2: edsp.tile([128, TPC * HEADS], F32, tag="es2", name="es2"),
                3: edsp.tile([128, TPC * 1], F32, tag="es3", name="es3"),
            }
            for _l in (1, 2, 3):
                # rows >= vr of the last tile stay uninitialized otherwise;
                # NaN garbage there poisons the eds matmul (0 * NaN = NaN)
                nc.vector.memset(ed_sb[_l][:], 0.0)
                nc.vector.memset(es_sb[_l][:], 0.0)

            # ---------- matmul phase (one dst tile)
            def mm_tile(layer, rt, tsh_w):
                if layer == 1:
                    nfc, Wm_sb, Wa_sb, Tsh, trow, hcols, nh = (
                        F_IN // 128, Wm1_sb, Wa1_sb, T1sh, TROW, HC, HEADS)
                elif layer == 2:
                    nfc, Wm_sb, Wa_sb, Tsh, trow, hcols, nh = (
                        IN2 // 128, Wm2_sb, Wa2_sb, T2sh, TROW, HC, HEADS)
                else:
                    nfc, Wm_sb, Wa_sb, Tsh, trow, hcols, nh = (
                        IN2 // 128, Wm3_sb, Wa3_sb, T3sh, TROW3, OUT, 1)
                acols = 2 * nh
                nxc = HC // 128

                r0 = rt * 128
                vr = min(128, R - r0)
                if vr <= 0:
                    return
                nxcf = F_IN // 128
                if True:
                    xtile = xtp.tile([128, nxcf * 128], BF16, tag="xtile")
                    for fx in range(nxcf):
                        nc.sync.dma_start(
                            out=xtile[:, fx * 128:(fx + 1) * 128],
                            in_=xT[(rt * nxcf + fx) * 128:
                                   (rt * nxcf + fx + 1) * 128, :])
                    pm = pmmp.tile([128, max(hcols, 8)], F32, tag="pm")
                    pa = pamp.tile([128, 16], F32, tag="pa")
                    for fc in range(nfc):
                        if layer == 1:
                            lhsT = xtile[:, fc * 128: fc * 128 + vr]
                        elif fc < nxc:
                            lhsT = outT_sb[:, fc * NTILE + r0:
                                           fc * NTILE + r0 + vr]
                        else:
                            fx = fc - nxc
                            lhsT = xtile[:, fx * 128: fx * 128 + vr]
                        nc.tensor.matmul(out=pm[:vr, :hcols], lhsT=lhsT,
                                         rhs=Wm_sb[:, fc * hcols:(fc + 1) * hcols],
                                         start=(fc == 0), stop=(fc == nfc - 1))
                        nc.tensor.matmul(out=pa[:vr, :acols], lhsT=lhsT,
                                         rhs=Wa_sb[:, fc * acols:(fc + 1) * acols],
                                         start=(fc == 0), stop=(fc == nfc - 1))
                    zrow = mmzp.tile([128, trow], BF16, tag="zrow")
                    if rt < 2:
                        nc.vector.memset(zrow[:, hcols + acols:], 0.0)
                    nc.vector.tensor_copy(out=zrow[:vr, :hcols],
                                          in_=pm[:vr, :hcols])
                    nc.vector.tensor_copy(
                        out=zrow[:vr, hcols:hcols + 2 * nh].bitcast(F32),
                        in_=pa[:vr, 0:nh])
                    nc.vector.tensor_copy(
                        out=es_sb[layer][:vr, rt * nh:(rt + 1) * nh],
                        in_=pa[:vr, 0:nh])
                    # ed -> resident SBUF bf16 table (scalar engine copy)
                    nc.scalar.activation(
                        out=ed_sb[layer][:vr, rt * nh:(rt + 1) * nh],
                        in_=pa[:vr, nh:2 * nh],
                        func=mybir.ActivationFunctionType.Copy)
                    tsh_w.append((rt, nc.sync.dma_start(
                        out=Tsh[r0:r0 + vr, :], in_=zrow[:vr, :])))

            # ---------- aggregation phase
            def agg_phase(layer, ccs, Tsh, tsh_w, post_pair=None):
                if layer == 3:
                    T_, trow, hcols, nh = T3, TROW3, OUT, 1
                    bias = bias_sb["b3"]
                else:
                    T_, trow, hcols, nh = (T1 if layer == 1 else T2), TROW, HC, HEADS
                    bias = bias_sb["b1"] if layer == 1 else bias_sb["b2"]
                esoff = hcols
                edt = ed_sb[layer]
                tshw_by_rt = dict(tsh_w)

                # batched self-loop weights: sx = exp(lrelu(es + ed))
                sx = sxp.tile([128, TPC * nh], BF16, tag="sx", name="sx")
                sxt = sxp.tile([128, TPC * nh], F32, tag="sxt", name="sxt")
                nc.vector.tensor_tensor(
                    out=sxt[:], in0=es_sb[layer][:, :TPC * nh],
                    in1=edt[:, :TPC * nh], op=mybir.AluOpType.add)
                sxt2 = sxp.tile([128, TPC * nh], F32, tag="sxt2", name="sxt2")
                nc.scalar.activation(
                    out=sxt2[:], in_=sxt[:],
                    func=mybir.ActivationFunctionType.Copy, scale=NEG_SLOPE)
                nc.vector.tensor_tensor(
                    out=sxt[:], in0=sxt[:], in1=sxt2[:],
                    op=mybir.AluOpType.max)
                nc.scalar.activation(
                    out=sx[:], in_=sxt[:],
                    func=mybir.ActivationFunctionType.Exp)

                for pair_i, pm_ in enumerate(pair_meta):
                    tiles = pm_["tiles"]
                    na_c, nb_c, nck = pm_["na_c"], pm_["nb_c"], pm_["nck"]
                    desc, m0 = pm_["desc"], pm_["m0"]
                    a0, b0 = pm_["a0"], pm_["b0"]
                    M = len(desc)
                    dodbg = DBG and layer == DBGL and pair_i == 0
                    dodbg_all = DBG and layer == DBGL

                    buf = gabp.tile([128, NCKMAX * trow], BF16, tag="buf")
                    gs = []
                    for cs in range(0, na_c, 8):
                        ck = min(8, na_c - cs)
                        gs.append(nc.gpsimd.dma_gather(
                            buf[:, cs * trow:(cs + ck) * trow].rearrange(
                                "p (c w) -> p c w", w=trow),
                            T_[:],
                            idxA_sb[:, (a0 + cs) * 8:(a0 + cs + ck) * 8],
                            ck * 128, numreg(ck * 128), trow))
                    for cs in range(0, nb_c, 8):
                        ck = min(8, nb_c - cs)
                        gs.append(nc.gpsimd.dma_gather(
                            buf[:, (na_c + cs) * trow:
                                (na_c + cs + ck) * trow].rearrange(
                                "p (c w) -> p c w", w=trow),
                            T_[ATH:],
                            idxB_sb[:, (b0 + cs) * 8:(b0 + cs + ck) * 8],
                            ck * 128, numreg(ck * 128), trow))
                    for g_ in gs:
                        for cc in ccs:
                            dep(g_, cc, "gather reads allgathered table")
                        for d_ in dummy_w[id(T_)]:
                            dep(g_, d_, "gather reads dummy rows")

                    # masks
                    mk = mkp.tile([128, MMAX * 128], BF16, tag="mk")
                    nc.sync.dma_start(
                        out=mk[:, :M * 128],
                        in_=msk_p[:, m0 * 128:(m0 + M) * 128])
                    mkT = mkTp.tile([128, MMAX * 128], BF16, tag="mkT")
                    nc.sync.dma_start(
                        out=mkT[:, :M * 128],
                        in_=mskT_p[:, m0 * 128:(m0 + M) * 128])

                    if dodbg:
                        nc.sync.dma_start(out=dbg["dbg_outT"][:, :],
                                          in_=outT_sb[:, :])
                        dT = nc.sync.dma_start(out=dbg["dbg_T1"][:, :trow],
                                               in_=T_[0:512, :])
                        for cc in ccs:
                            dep(dT, cc, "dbg reads table")
                        nc.sync.dma_start(out=dbg["dbg_buf"][:, :],
                                          in_=buf[:, :8 * trow])
                        nc.sync.dma_start(out=dbg["dbg_mk"][:, :],
                                          in_=mk[:, :8 * 128])
                        nc.sync.dma_start(out=dbg["dbg_mkT"][:, :],
                                          in_=mkT[:, :8 * 128])
                        nc.sync.dma_start(out=dbg["dbg_ed"][:, :],
                                          in_=ed_sb[1][:, :])

                    # eds: per chunk, sum over descriptors of mskT @ ed_tile
                    eds_ps = psEp.tile([128, max(NCKMAX * nh, 8)], F32,
                                       tag="eds")
                    by_chunk = {}
                    for m, (j, ti) in enumerate(desc):
                        by_chunk.setdefault(j, []).append((m, ti))
                    for j, ms in sorted(by_chunk.items()):
                        for q, (m, ti) in enumerate(ms):
                            tt = tiles[ti]
                            nc.tensor.matmul(
                                out=eds_ps[:, j * nh:(j + 1) * nh],
                                lhsT=mkT[:, m * 128:(m + 1) * 128],
                                rhs=edt[:, tt * nh:(tt + 1) * nh],
                                start=(q == 0), stop=(q == len(ms) - 1),
                                skip_group_check=True)

                    # logits: et = es + eds ; ex = exp(lrelu(et))
                    bv = buf[:].rearrange("p (c w) -> p c w", w=trow)
                    et = lgtp.tile([128, NCKMAX * nh], F32, tag="et")
                    nc.vector.tensor_tensor(
                        out=et[:, :nck * nh].rearrange(
                            "p (c h) -> p c h", h=nh),
                        in0=bv[:, 0:nck, esoff:esoff + 2 * nh].bitcast(F32),
                        in1=eds_ps[:, :nck * nh].rearrange(
                            "p (c h) -> p c h", h=nh),
                        op=mybir.AluOpType.add)
                    et2 = lgtp.tile([128, NCKMAX * nh], F32, tag="et2")
                    nc.scalar.activation(
                        out=et2[:, :nck * nh], in_=et[:, :nck * nh],
                        func=mybir.ActivationFunctionType.Copy,
                        scale=NEG_SLOPE)
                    nc.vector.tensor_tensor(
                        out=et2[:, :nck * nh], in0=et[:, :nck * nh],
                        in1=et2[:, :nck * nh], op=mybir.AluOpType.max)
                    ex = lgtp.tile([128, NCKMAX * nh], BF16, tag="ex")
                    nc.scalar.activation(
                        out=ex[:, :nck * nh], in_=et2[:, :nck * nh],
                        func=mybir.ActivationFunctionType.Exp)
                    if dodbg:
                        edscp = epp.tile([128, 32 * 8], F32, tag="edscp")
                        nc.vector.tensor_copy(out=edscp[:, :nck * nh],
                                              in_=eds_ps[:, :nck * nh])
                        nc.sync.dma_start(out=dbg["dbg_eds"][:, :],
                                          in_=edscp[:, :])
                        nc.sync.dma_start(out=dbg["dbg_et"][:, :nck * nh],
                                          in_=et[:, :nck * nh])
                        nc.sync.dma_start(out=dbg["dbg_ex"][:, :nck * nh],
                                          in_=ex[:, :nck * nh])

                    # scaled messages (in halves to bound SBUF)
                    pag = [psAp.tile([128, max(hcols, 8)], F32, tag="pag",
                                     name=f"pag{i}") for i in range(len(tiles))]
                    den = psDenp.tile([128, 16], F32, tag="den")
                    HALFMAX = (NCKMAX + 1) // 2
                    half = (nck + 1) // 2
                    mp_halves = []
                    for hi, h0 in enumerate(range(0, nck, half)):
                        h1 = min(h0 + half, nck)
                        mp_ = mpp.tile([128, HALFMAX * hcols], BF16,
                                       tag="mp", name=f"mp{hi}")
                        nc.vector.tensor_tensor(
                            out=mp_[:, :(h1 - h0) * hcols].rearrange(
                                "p (c h k) -> p c h k", h=nh, k=hcols // nh),
                            in0=bv[:, h0:h1, 0:hcols].rearrange(
                                "p c (h k) -> p c h k", h=nh),
                            in1=ex[:, h0 * nh:h1 * nh].rearrange(
                                "p (c h) -> p c h", h=nh)[:, :, :, None]
                                .to_broadcast(
                                    [128, h1 - h0, nh, hcols // nh]),
                            op=mybir.AluOpType.mult)
                        mp_halves.append((h0, h1, mp_))
                    # accumulation matmuls, tile-by-tile so each PSUM
                    # accumulation chain opens and closes sequentially
                    by_tile = {}
                    for m, (j, ti) in enumerate(desc):
                        by_tile.setdefault(ti, []).append((m, j))
                    for ti, ms in sorted(by_tile.items()):
                        for q, (m, j) in enumerate(ms):
                            st = (q == 0)
                            sp = (q == len(ms) - 1)
                            h0, h1, mp_ = next(
                                t for t in mp_halves
                                if t[0] <= j < t[1])
                            nc.tensor.matmul(
                                out=pag[ti][:, :hcols],
                                lhsT=mk[:, m * 128:(m + 1) * 128],
                                rhs=mp_[:, (j - h0) * hcols:
                                        (j - h0 + 1) * hcols],
                                start=st, stop=sp, skip_group_check=True)
                        for q, (m, j) in enumerate(ms):
                            nc.tensor.matmul(
                                out=den[:, ti * 8:ti * 8 + nh],
                                lhsT=mk[:, m * 128:(m + 1) * 128],
                                rhs=ex[:, j * nh:(j + 1) * nh],
                                start=(q == 0), stop=(q == len(ms) - 1),
                                skip_group_check=True)

                    # epilogue per tile (folds in the local self-loop term)
                    if dodbg_all:
                        dencp = epp.tile([128, 16], F32, tag="dencp")
                        nc.vector.tensor_copy(out=dencp[:], in_=den[:])
                        nc.sync.dma_start(
                            out=dbg["dbg_denall"][pair_i * 128:
                                                  (pair_i + 1) * 128, :],
                            in_=dencp[:, :])
                    for ti, tt in enumerate(tiles):
                        r0 = tt * 128
                        vr = min(128, R - r0)
                        if vr <= 0:
                            continue
                        zt = ztp.tile([128, max(hcols, 8)], BF16, tag="zt")
                        ztd = nc.sync.dma_start(out=zt[:vr, :hcols],
                                                in_=Tsh[r0:r0 + vr, 0:hcols])
                        dep(ztd, tshw_by_rt[tt], "self z reads shard write")
                        selfmp = epp.tile([128, max(hcols, 8)], F32,
                                          tag="selfmp")
                        nc.vector.tensor_tensor(
                            out=selfmp[:, :hcols].rearrange(
                                "p (h k) -> p h k", h=nh),
                            in0=zt[:, :hcols].rearrange(
                                "p (h k) -> p h k", h=nh),
                            in1=sx[:, tt * nh:(tt + 1) * nh, None]
                                .to_broadcast([128, nh, hcols // nh]),
                            op=mybir.AluOpType.mult)
                        num = epp.tile([128, max(hcols, 8)], F32, tag="num")
                        nc.vector.tensor_tensor(
                            out=num[:, :hcols], in0=pag[ti][:, :hcols],
                            in1=selfmp[:, :hcols], op=mybir.AluOpType.add)
                        dent = epp.tile([128, 8], F32, tag="dent")
                        nc.vector.tensor_tensor(
                            out=dent[:, :nh], in0=den[:, ti * 8:ti * 8 + nh],
                            in1=sx[:, tt * nh:(tt + 1) * nh],
                            op=mybir.AluOpType.add)
                        rden = epp.tile([128, 8], F32, tag="rden")
                        nc.vector.reciprocal(out=rden[:, :nh],
                                             in_=dent[:, :nh])
                        o1 = epp.tile([128, max(hcols, 8)], F32, tag="o1")
                        nc.vector.tensor_tensor(
                            out=o1[:, :hcols].rearrange(
                                "p (h k) -> p h k", h=nh),
                            in0=num[:, :hcols].rearrange(
                                "p (h k) -> p h k", h=nh),
                            in1=rden[:, :nh, None].to_broadcast(
                                [128, nh, hcols // nh]),
                            op=mybir.AluOpType.mult)
                        nc.vector.tensor_tensor(
                            out=o1[:, :hcols], in0=o1[:, :hcols],
                            in1=bias[:, :], op=mybir.AluOpType.add)
                        if dodbg_all and ti == 0:
                            nc.sync.dma_start(
                                out=dbg["dbg_o1all"][pair_i * 128:
                                                     pair_i * 128 + 128,
                                                     :hcols],
                                in_=o1[:, :hcols])
                        if layer != 3:
                            ob = epp.tile([128, hcols], BF16, tag="ob")
                            nc.scalar.activation(
                                out=ob[:, :], in_=o1[:, :hcols],
                                func=mybir.ActivationFunctionType.Relu)
                            pt = psTp.tile([128, (HC // 128) * 128], BF16,
                                           tag="pt")
                            for q in range(hcols // 128):
                                nc.tensor.transpose(
                                    out=pt[:, q * 128:q * 128 + vr],
                                    in_=ob[:vr, q * 128:(q + 1) * 128],
                                    identity=ident[:vr, :vr])
                            for q in range(hcols // 128):
                                nc.scalar.activation(
                                    out=outT_sb[:, q * NTILE + r0:
                                                q * NTILE + r0 + vr],
                                    in_=pt[:, q * 128:q * 128 + vr],
                                    func=mybir.ActivationFunctionType.Copy)
                        else:
                            mx = epp.tile([128, 1], F32, tag="mx")
                            nc.vector.tensor_reduce(
                                out=mx[:], in_=o1[:, :hcols],
                                op=mybir.AluOpType.max,
                                axis=mybir.AxisListType.X)
                            zc = epp.tile([128, hcols], F32, tag="zc")
                            nc.vector.tensor_scalar(
                                out=zc[:], in0=o1[:, :hcols], scalar1=mx[:],
                                scalar2=None,
                                op0=mybir.AluOpType.subtract)
                            ex3 = epp.tile([128, hcols], F32, tag="ex3")
                            s3 = epp.tile([128, 1], F32, tag="s3")
                            nc.scalar.activation(
                                out=ex3[:], in_=zc[:],
                                func=mybir.ActivationFunctionType.Exp,
                                accum_out=s3[:])
                            ln3 = epp.tile([128, 1], F32, tag="ln3")
                            nc.scalar.activation(
                                out=ln3[:], in_=s3[:],
                                func=mybir.ActivationFunctionType.Ln)
                            res = epp.tile([128, hcols], F32, tag="res")
                            nc.vector.tensor_scalar(
                                out=res[:], in0=zc[:], scalar1=ln3[:],
                                scalar2=None,
                                op0=mybir.AluOpType.subtract)
                            nc.sync.dma_start(out=out_ext[r0:r0 + vr, :],
                                              in_=res[:vr, :])
                    if post_pair is not None:
                        post_pair(pair_i, tiles)

            # ---------- the three layers, software-pipelined: layer l+1's
            # matmul tiles and AllGather chunks are emitted inside layer l's
            # aggregation pair loop so they execute during it.
            tables = {1: (T1sh, T1), 2: (T2sh, T2), 3: (T3sh, T3)}
            tshw = {1: [], 2: [], 3: []}
            ccsd = {1: [], 2: [], 3: []}

            def emit_ag(layer, k):
                Tsh_, T_ = tables[layer]
                t0, t1, off_k, rows_k = ag_chunks[k]
                r0 = t0 * 128
                cc = nc.gpsimd.collective_compute(
                    "AllGather",
                    mybir.AluOpType.bypass,
                    replica_groups=[list(range(P))],
                    ins=[Tsh_[r0:r0 + rows_k, :]],
                    outs=[T_[1 + off_k:1 + off_k + P * rows_k, :]],
                )
                for (rt, w_) in tshw[layer]:
                    if t0 <= rt < t1:
                        dep(cc, w_, "allgather reads shard chunk writes")
                ccsd[layer].append(cc)

            def make_post_pair(next_layer):
                def post_pair(pair_i, tiles):
                    for rt in tiles:
                        mm_tile(next_layer, rt, tshw[next_layer])
                    done = tiles[-1] + 1
                    for k, (t0, t1, off_k, rows_k) in enumerate(ag_chunks):
                        if done == t1 or (done == TPC and t1 >= TPC):
                            emit_ag(next_layer, k)
                return post_pair

            for rt in range(TPC):
                mm_tile(1, rt, tshw[1])
            for k in range(len(ag_chunks)):
                emit_ag(1, k)
            agg_phase(1, ccsd[1], T1sh, tshw[1], make_post_pair(2))
            agg_phase(2, ccsd[2], T2sh, tshw[2], make_post_pair(3))
            agg_phase(3, ccsd[3], T3sh, tshw[3])

    _regstack.close()
    from concourse.library_overlay import lower_extended_insts
    lower_extended_insts(nc)
    return nc


# --------------------------------------------------------------------------
# host wrapper
# --------------------------------------------------------------------------
def _prep_inputs(inputs, cfg):
    N, P, R, TPC = cfg["N"], cfg["P"], cfg["R"], cfg["TPC"]
    HEADS, HID, OUT, F_IN = cfg["HEADS"], cfg["HID"], cfg["OUT"], cfg["F_IN"]
    HC = HEADS * HID

    x = np.asarray(inputs["x"], np.float32)
    edge_index = np.asarray(inputs["edge_index"], np.int64)

    tile_ranges = cfg["TILE_RANGES"]
    perm, offs, rows_ks = build_perm(N, P, R, tile_ranges)
    ag_chunks = [(t0, t1, offs[k], rows_ks[k])
                 for k, (t0, t1) in enumerate(tile_ranges)]
    shards, pair_meta, CA, CB, Mtot = build_edges2(
        edge_index, N, P, R, TPC, cfg["ATH"], perm)

    def fold(W, a_s, a_d, heads, ch):
        F = W.shape[0]
        Wr = W.reshape(F, heads, ch)
        Wa = np.zeros((F, 2 * heads), np.float32)
        for h in range(heads):
            Wa[:, h] = Wr[:, h] @ a_s[h]
            Wa[:, heads + h] = Wr[:, h] @ a_d[h]
        return Wa

    w1 = np.asarray(inputs["w1"], np.float32)
    w2 = np.asarray(inputs["w2"], np.float32)
    w3 = np.asarray(inputs["w3"], np.float32)
    Wa1 = fold(w1, np.asarray(inputs["a1s"]), np.asarray(inputs["a1d"]),
               HEADS, HID)
    Wa2 = fold(w2, np.asarray(inputs["a2s"]), np.asarray(inputs["a2d"]),
               HEADS, HID)
    Wa3 = fold(w3, np.asarray(inputs["a3s"]), np.asarray(inputs["a3d"]),
               1, OUT)

    dum640 = np.zeros((1, cfg["TROW"]), BF16NP)
    dum640.view(np.uint8)[0, 2 * HC:2 * HC + HEADS * 4] = \
        np.full(HEADS, NEG_BIG, np.float32).view(np.uint8)
    dum128 = np.zeros((1, cfg["TROW3"]), BF16NP)
    dum128.view(np.uint8)[0, 2 * OUT:2 * OUT + 4] = \
        np.frombuffer(np.float32(NEG_BIG).tobytes(), np.uint8)

    common = {
        "Wm1": w1.astype(BF16NP), "Wa1": Wa1.astype(BF16NP),
        "Wm2": w2.astype(BF16NP), "Wa2": Wa2.astype(BF16NP),
        "Wm3": w3.astype(BF16NP), "Wa3": Wa3.astype(BF16NP),
        "b1": np.tile(np.asarray(inputs["b1"], np.float32).reshape(1, HC),
                      (128, 1)),
        "b2": np.tile(np.asarray(inputs["b2"], np.float32).reshape(1, HC),
                      (128, 1)),
        "b3": np.tile(np.asarray(inputs["b3"], np.float32).reshape(1, OUT),
                      (128, 1)),
        "dum640": dum640, "dum128": dum128,
    }
    TPC_ = cfg["TPC"]
    nxcf = F_IN // 128
    in_maps = []
    for p in range(P):
        m = dict(common)
        # tiled partition-contiguous layout: block (rt, fc) = [128 feat,
        # 128 nodes] contiguous, so each tile load is one linear 32KB read
        xp = np.zeros((TPC_ * 128, F_IN), np.float32)
        xp[:R] = x[p * R:(p + 1) * R, :]
        xt4 = xp.reshape(TPC_, 128, nxcf, 128).transpose(0, 2, 3, 1)
        m["xT"] = np.ascontiguousarray(
            xt4.reshape(TPC_ * nxcf * 128, 128)).astype(BF16NP)
        m["idxA"] = shards[p]["idxA"]
        m["idxB"] = shards[p]["idxB"]
        m["msk"] = shards[p]["msk"]
        m["mskT"] = shards[p]["mskT"]
        in_maps.append(m)
    return in_maps, pair_meta, CA, CB, Mtot, ag_chunks


def default_cfg():
    return dict(N=50000, P=8, R=6250, TPC=49, F_IN=256, HID=64, HEADS=8,
                OUT=16, ATH=32768, TROW=640, TROW3=128,
                NEG_SLOPE=0.2,
                TILE_RANGES=[(0, 12), (12, 24), (24, 36), (36, 49)])


def kernel(**inputs):
    cfg = default_cfg()
    in_maps, pair_meta, CA, CB, Mtot, ag_chunks = _prep_inputs(inputs, cfg)
    nc = build_program(cfg, pair_meta, CA, CB, Mtot, ag_chunks)

    _split_multiwait(nc)
    from concourse.bass_utils import run_bass_kernel_spmd
    trace = bool(os.environ.get("GNN_TRACE"))
    if trace:
        sys.path.insert(0, "/root/problem/work")
        import axonhook  # noqa
    res = run_bass_kernel_spmd(nc, in_maps, list(range(cfg["P"])),
                               trace=trace)
    if trace:
        kernel.last_exec_ns = res.exec_time_ns
    if os.environ.get("GNN_DEBUG"):
        np.savez("/root/problem/work/dbg.npz",
                 **{k: np.asarray(v) for k, v in res.results[0].items()
                    if k.startswith("dbg_")})
    out = np.concatenate([res.results[p]["out"] for p in range(cfg["P"])],
                         axis=0)
    return out.astype(np.float32)


# revision 46
# speedup vs baseline: 1.1546x; 1.1546x over previous
"""3-layer GAT (gnn_message_passing) on 8 Trainium2 NeuronCores.

Strategy: nodes sharded by dst octant.  Per layer:
  - node-parallel matmul phase computes Z=[h|es] rows for the core's 6250
    nodes plus a per-tile ed table kept in SBUF
  - chunked AllGather of the Z table (bf16 rows, 1280B) overlapped with the
    matmul phase via a chunk-major permuted table row layout
  - dst-pair aggregation: dma_gather edge messages by src id into a merged
    A|B buffer, ed broadcast to slots via mskT@ed tensor matmuls
    (host-precomputed transposed selection masks, streamed), logits
    exp(lrelu(es+ed)) on the scalar engine, message scaling on vector,
    host-precomputed selection-mask matmuls accumulate numerator+denominator
    in PSUM across all chunk segments, epilogue normalizes + bias (+relu)
    and transposes into the next layer's matmul input.
Edge structure (sorting by (pair, A/B group, tile, dst), chunk-aligned
A/B regions padded to the cross-core max, selection masks and their
transposes) is precomputed on host; dummy table rows carry es=-1e9 so
padded slots get exp(-inf)=0 weight.
"""
import os
import sys

sys.path.insert(0, "/opt/trn_rl_repo")

import numpy as np
import ml_dtypes

BF16NP = ml_dtypes.bfloat16
NEG_BIG = -1.0e9

MAX_WAITS = 1


def _split_multiwait(nc):
    """walrus in this env rejects >1 sync-wait per instruction: split excess
    waits onto same-engine NoOps."""
    import concourse.mybir as mybir
    for _name, bbb in nc.bb_map.items():
        il = bbb.bb.instructions
        new = []
        changed = False
        for inst in il:
            si = getattr(inst, "sync_info", None)
            ow = list(si.on_wait) if si is not None and si.on_wait else []
            if len(ow) > MAX_WAITS:
                excess, keep = ow[:-MAX_WAITS], ow[-MAX_WAITS:]
                for j, w in enumerate(excess):
                    new.append(mybir.InstNoOp(
                        name=f"{inst.name}_sw{j}",
                        engine=inst.engine,
                        bass_nofuse=True,
                        sync_info=mybir.SyncInfo(on_wait=[w], on_update=[]),
                    ))
                inst.sync_info = mybir.SyncInfo(
                    on_wait=keep, on_update=list(si.on_update))
                changed = True
            new.append(inst)
        if changed:
            bbb.bb.instructions = new


# --------------------------------------------------------------------------
# host-side edge preprocessing
# --------------------------------------------------------------------------
def wrap16(vals):
    """dma_gather index layout: element i -> [i%16, i//16]; [128, n//16]."""
    v = np.asarray(vals, dtype=np.int64)
    num = len(v)
    assert num % 16 == 0
    a = np.zeros((16, num // 16), dtype=np.int16)
    i = np.arange(num)
    a[i % 16, i // 16] = v.astype(np.int16)
    return np.tile(a, (8, 1))


def build_perm(N, P, R, tile_ranges):
    """Chunk-major table row permutation.  Table rows (1-based, dummies at 0
    and N+1): row = 1 + off_k + oct*rows_k + (loc - r0_k)."""
    TPC = (R + 127) // 128
    perm = np.zeros(N, np.int64)
    off = 0
    offs = []
    rows_ks = []
    for (t0, t1) in tile_ranges:
        r0 = t0 * 128
        r1 = min(t1 * 128, R)
        rows_k = r1 - r0
        offs.append(off)
        rows_ks.append(rows_k)
        off += P * rows_k
    node = np.arange(N)
    oct_ = node // R
    loc = node % R
    for k, (t0, t1) in enumerate(tile_ranges):
        r0 = t0 * 128
        r1 = min(t1 * 128, R)
        sel = (loc >= r0) & (loc < r1)
        perm[sel] = 1 + offs[k] + oct_[sel] * rows_ks[k] + (loc[sel] - r0)
    return perm, offs, rows_ks


def build_edges2(edge_index, N, P, R, TPC, ATH, perm):
    """Per-core gather index streams + per-pair metadata + selection masks.

    Per pair (2 dst tiles): A region (perm row < ATH) then B region, each
    chunk-aligned (128 slots), sizes = max over cores.  Masks are emitted per
    host-shared (chunk, tile) descriptor; per-core mask data zero-filled
    where a core has no slots.
    """
    # self-loops are handled locally in the epilogue, not gathered
    src = edge_index[0].astype(np.int64)
    dst = edge_index[1].astype(np.int64)
    owner = dst // R
    NPAIR = (TPC + 1) // 2

    # per (core, pair): sorted slot lists
    core_pair = [[None] * NPAIR for _ in range(P)]
    nA = np.zeros((P, NPAIR), np.int64)
    nB = np.zeros((P, NPAIR), np.int64)
    for p in range(P):
        sel = np.nonzero(owner == p)[0]
        d = dst[sel] - p * R
        row = perm[src[sel]]
        grp = (row >= ATH).astype(np.int64)
        t = d // 128
        pair = t // 2
        order = np.lexsort((d, t, grp, pair))
        d = d[order]; row = row[order]; grp = grp[order]
        t = t[order]; pair = pair[order]
        bounds = np.searchsorted(pair, np.arange(NPAIR + 1))
        for g in range(NPAIR):
            s0, s1 = bounds[g], bounds[g + 1]
            gg = grp[s0:s1]
            na = int(np.searchsorted(gg, 1))
            core_pair[p][g] = (d[s0:s1], row[s0:s1], t[s0:s1], na)
            nA[p, g] = na
            nB[p, g] = (s1 - s0) - na

    # shared per-pair chunk counts
    nA128 = (nA.max(axis=0) + 127) // 128
    nB128 = (nB.max(axis=0) + 127) // 128
    nA128 = np.maximum(nA128, 1)
    nB128 = np.maximum(nB128, 1)

    pair_meta = []
    CA = CB = 0
    Mtot = 0
    for g in range(NPAIR):
        tiles = [2 * g] + ([2 * g + 1] if 2 * g + 1 < TPC else [])
        na_c = int(nA128[g])
        nb_c = int(nB128[g])
        nck = na_c + nb_c
        # union descriptors (j, ti): chunk j intersects tile ti on any core
        desc = []
        seen = set()
        for p in range(P):
            d, row, t, na = core_pair[p][g]
            for (lo, n, base_ck) in ((0, na, 0), (na, len(d) - na, na_c)):
                tt = t[lo:lo + n]
                for ti in range(len(tiles)):
                    idx = np.nonzero(tt == tiles[ti])[0]
                    if len(idx) == 0:
                        continue
                    c0 = base_ck + idx[0] // 128
                    c1 = base_ck + idx[-1] // 128
                    for j in range(c0, c1 + 1):
                        seen.add((j, ti))
        desc = sorted(seen)
        pair_meta.append(dict(tiles=tiles, na_c=na_c, nb_c=nb_c, nck=nck,
                              desc=desc, m0=Mtot, a0=CA, b0=CB))
        CA += na_c
        CB += nb_c
        Mtot += len(desc)

    out = []
    for p in range(P):
        idxA_parts = []
        idxB_parts = []
        msk = np.zeros((Mtot * 128, 128), np.float32)   # [slotblock, d] rows
        for g in range(NPAIR):
            pm = pair_meta[g]
            d, row, t, na = core_pair[p][g]
            na_c, nb_c = pm["na_c"], pm["nb_c"]
            va = np.zeros(na_c * 128, np.int64)          # pad -> dummy row 0
            va[:na] = row[:na]
            vb = np.full(nb_c * 128, N + 1 - ATH, np.int64)
            vb[:len(d) - na] = row[na:] - ATH
            idxA_parts.append(va)
            idxB_parts.append(vb)
            # slot -> (chunk, within) ; chunk = A chunks then B chunks
            slot_ck = np.concatenate([
                np.arange(na) // 128,
                na_c + np.arange(len(d) - na) // 128])
            slot_in = np.concatenate([
                np.arange(na) % 128, np.arange(len(d) - na) % 128])
            dloc = d - (t * 128)
            for m, (j, ti) in enumerate(pm["desc"]):
                sel = np.nonzero((slot_ck == j) & (t == pm["tiles"][ti]))[0]
                if len(sel) == 0:
                    continue
                blk = (pm["m0"] + m) * 128
                msk[blk + slot_in[sel], dloc[sel]] = 1.0
        idxA = wrap16(np.concatenate(idxA_parts))
        idxB = wrap16(np.concatenate(idxB_parts))
        # msk view [Mtot, 128slot, 128d] -> device msk [128slot, Mtot*128d]
        m3 = msk.reshape(Mtot, 128, 128)
        mskd = np.ascontiguousarray(
            m3.transpose(1, 0, 2).reshape(128, Mtot * 128)).astype(BF16NP)
        mskT = np.ascontiguousarray(
            m3.transpose(2, 0, 1).reshape(128, Mtot * 128)).astype(BF16NP)
        out.append({"idxA": idxA, "idxB": idxB, "msk": mskd, "mskT": mskT})
    return out, pair_meta, CA, CB, Mtot


# --------------------------------------------------------------------------
# device program
# --------------------------------------------------------------------------
def build_program(cfg, pair_meta, CA, CB, Mtot, ag_chunks):
    import concourse.bass as bass
    import concourse.mybir as mybir
    import concourse.tile as tile
    from concourse.library_config import mlp
    from concourse.masks import make_identity
    from concourse.tile_rust import add_dep_helper

    def _mi(x):
        return getattr(x, "ins", x)

    def dep(a, b, why):
        add_dep_helper(_mi(a), _mi(b), reason=why)

    F32 = mybir.dt.float32
    BF16 = mybir.dt.bfloat16
    I16 = mybir.dt.int16

    N, P, R, TPC = cfg["N"], cfg["P"], cfg["R"], cfg["TPC"]
    F_IN, HID, HEADS, OUT = cfg["F_IN"], cfg["HID"], cfg["HEADS"], cfg["OUT"]
    ATH = cfg["ATH"]
    HC = HID * HEADS
    IN2 = HC + F_IN
    TROW = cfg["TROW"]
    TROW3 = cfg["TROW3"]
    NRT = N + 2
    NTILE = TPC * 128
    NCKMAX = max(pm["nck"] for pm in pair_meta)
    MMAX = max(len(pm["desc"]) for pm in pair_meta)
    NEG_SLOPE = cfg["NEG_SLOPE"]

    nc = bass.Bass()

    ps = {}

    def par(name, shape, dt):
        ps[name] = nc.declare_dram_parameter(name, list(shape), dt,
                                             isOutput=False)
        return ps[name]

    xT = par("xT", [TPC * (F_IN // 128) * 128, 128], BF16)
    Wm1 = par("Wm1", [F_IN, HC], BF16)
    Wa1 = par("Wa1", [F_IN, 2 * HEADS], BF16)
    Wm2 = par("Wm2", [IN2, HC], BF16)
    Wa2 = par("Wa2", [IN2, 2 * HEADS], BF16)
    Wm3 = par("Wm3", [IN2, OUT], BF16)
    Wa3 = par("Wa3", [IN2, 2], BF16)
    b1 = par("b1", [128, HC], F32)
    b2 = par("b2", [128, HC], F32)
    b3 = par("b3", [128, OUT], F32)
    idxA_p = par("idxA", [128, CA * 8], I16)
    idxB_p = par("idxB", [128, CB * 8], I16)
    msk_p = par("msk", [128, Mtot * 128], BF16)
    mskT_p = par("mskT", [128, Mtot * 128], BF16)
    dum640 = par("dum640", [1, TROW], BF16)
    dum128 = par("dum128", [1, TROW3], BF16)
    out_ext = nc.declare_dram_parameter("out", [R, OUT], F32, isOutput=True)

    DBGL = int(os.environ.get("GNN_DEBUG", "0"))
    DBG = DBGL > 0
    dbg = {}
    if DBG:
        NPAIR = len(pair_meta)
        for nm, shape, dt in (
                ("dbg_T1", [512, TROW], BF16),
                ("dbg_buf", [128, 8 * TROW], BF16),
                ("dbg_mk", [128, 8 * 128], BF16),
                ("dbg_mkT", [128, 8 * 128], BF16),
                ("dbg_et", [128, 32 * 8], F32),
                ("dbg_ex", [128, 32 * 8], BF16),
                ("dbg_eds", [128, 32 * 8], F32),
                ("dbg_denall", [NPAIR * 128, 16], F32),
                ("dbg_o1all", [NPAIR * 128, 512], F32),
                ("dbg_ed", [128, 49 * 8], BF16),
                ("dbg_outT", [128, 4 * 6272], BF16),
        ):
            dbg[nm] = nc.declare_dram_parameter(nm, shape, dt, isOutput=True)

    T1 = nc.dram_tensor("T1", [NRT, TROW], BF16, addr_space="Shared")
    T2 = nc.dram_tensor("T2", [NRT, TROW], BF16, addr_space="Shared")
    T3 = nc.dram_tensor("T3", [NRT, TROW3], BF16, addr_space="Shared")
    T1sh = nc.dram_tensor("T1sh", [R, TROW], BF16)
    T2sh = nc.dram_tensor("T2sh", [R, TROW], BF16)
    T3sh = nc.dram_tensor("T3sh", [R, TROW3], BF16)

    nc.gpsimd.load_library(mlp)

    from contextlib import ExitStack
    _regstack = ExitStack()
    _regcache = {}

    def numreg(v):
        if v not in _regcache:
            r = _regstack.enter_context(nc.gpsimd.register(f"nidx{v}"))
            nc.gpsimd.reg_mov(r, v)
            _regcache[v] = r
        return _regcache[v]

    with tile.TileContext(nc) as tc:
        with ExitStack() as _pools:
            ep_ = _pools.enter_context
            constp = ep_(tc.tile_pool(name="const", bufs=1))
            wp = ep_(tc.tile_pool(name="w", bufs=1))
            xtp = ep_(tc.tile_pool(name="xt", bufs=8))
            outTp = ep_(tc.tile_pool(name="outT", bufs=1))
            edsp = ep_(tc.tile_pool(name="eds", bufs=1))
            mmzp = ep_(tc.tile_pool(name="mmz", bufs=4))
            gabp = ep_(tc.tile_pool(name="gab", bufs=2))
            mkp = ep_(tc.tile_pool(name="mk", bufs=2))
            mkTp = ep_(tc.tile_pool(name="mkT", bufs=2))
            lgtp = ep_(tc.tile_pool(name="lgt", bufs=2))
            mpp = ep_(tc.tile_pool(name="mp", bufs=2))
            epp = ep_(tc.tile_pool(name="ep", bufs=2))
            psAp = ep_(tc.tile_pool(name="psA", bufs=2, space="PSUM"))
            psDenp = ep_(tc.tile_pool(name="psDen", bufs=1, space="PSUM"))
            psEp = ep_(tc.tile_pool(name="psE", bufs=1, space="PSUM"))
            psTp = ep_(tc.tile_pool(name="psT", bufs=1, space="PSUM"))
            pmmp = ep_(tc.tile_pool(name="pmm", bufs=2, space="PSUM"))
            pamp = ep_(tc.tile_pool(name="pam", bufs=1, space="PSUM"))
            sxp = ep_(tc.tile_pool(name="sx", bufs=1))
            ztp = ep_(tc.tile_pool(name="zt", bufs=2))
            # ---------- constants / resident data
            ident = constp.tile([128, 128], BF16, tag="ident")
            make_identity(nc, ident[:])

            idxA_sb = constp.tile([128, CA * 8], I16, tag="idxA")
            nc.sync.dma_start(out=idxA_sb[:], in_=idxA_p[:])
            idxB_sb = constp.tile([128, CB * 8], I16, tag="idxB")
            nc.sync.dma_start(out=idxB_sb[:], in_=idxB_p[:])

            bias_sb = {}
            for nm, p_, w_ in (("b1", b1, HC), ("b2", b2, HC), ("b3", b3, OUT)):
                bias_sb[nm] = constp.tile([128, w_], F32, tag=nm, name=nm)
                nc.sync.dma_start(out=bias_sb[nm][:], in_=p_[:])

            dummy_w = {}
            for T_, dum in ((T1, dum640), (T2, dum640), (T3, dum128)):
                i1 = nc.sync.dma_start(out=T_[0:1, :], in_=dum[:])
                i2 = nc.sync.dma_start(out=T_[N + 1:N + 2, :], in_=dum[:])
                dummy_w[id(T_)] = [i1, i2]

            def load_w(p_, rows, cols, tag):
                nchunks = (rows + 127) // 128
                tl = wp.tile([128, nchunks * cols], BF16, tag=tag)
                for fc in range(nchunks):
                    r0 = fc * 128
                    vr = min(128, rows - r0)
                    nc.sync.dma_start(out=tl[:vr, fc * cols:(fc + 1) * cols],
                                      in_=p_[r0:r0 + vr, :])
                return tl

            Wm1_sb = load_w(Wm1, F_IN, HC, "Wm1")
            Wa1_sb = load_w(Wa1, F_IN, 2 * HEADS, "Wa1")
            Wm2_sb = load_w(Wm2, IN2, HC, "Wm2")
            Wa2_sb = load_w(Wa2, IN2, 2 * HEADS, "Wa2")
            Wm3_sb = load_w(Wm3, IN2, OUT, "Wm3")
            Wa3_sb = load_w(Wa3, IN2, 2, "Wa3")

            outT_sb = outTp.tile([128, (HC // 128) * NTILE], BF16, tag="outT")
            # per-layer ed tables (bf16), written by matmul phase
            ed_sb = {
                1: edsp.tile([128, TPC * HEADS], BF16, tag="ed1", name="ed1"),
                2: edsp.tile([128, TPC * HEADS], BF16, tag="ed2", name="ed2"),
                3: edsp.tile([128, TPC * 1], BF16, tag="ed3", name="ed3"),
            }
            es_sb = {
                1: edsp.tile([128, TPC * HEADS], F32, tag="es1", name="es1"),
                2: edsp.tile([128, TPC * HEADS], F32, tag="es2", name="es2"),
                3: edsp.tile([128, TPC * 1], F32, tag="es3", name="es3"),
            }
            for _l in (1, 2, 3):
                # rows >= vr of the last tile stay uninitialized otherwise;
                # NaN garbage there poisons the eds matmul (0 * NaN = NaN)
                nc.vector.memset(ed_sb[_l][:], 0.0)
                nc.vector.memset(es_sb[_l][:], 0.0)

            # ---------- matmul phase (one dst tile)
            def mm_tile(layer, rt, tsh_w):
                if layer == 1:
                    nfc, Wm_sb, Wa_sb, Tsh, trow, hcols, nh = (
                        F_IN // 128, Wm1_sb, Wa1_sb, T1sh, TROW, HC, HEADS)
                elif layer == 2:
                    nfc, Wm_sb, Wa_sb, Tsh, trow, hcols, nh = (
                        IN2 // 128, Wm2_sb, Wa2_sb, T2sh, TROW, HC, HEADS)
                else:
                    nfc, Wm_sb, Wa_sb, Tsh, trow, hcols, nh = (
                        IN2 // 128, Wm3_sb, Wa3_sb, T3sh, TROW3, OUT, 1)
                acols = 2 * nh
                nxc = HC // 128

                r0 = rt * 128
                vr = min(128, R - r0)
                if vr <= 0:
                    return
                nxcf = F_IN // 128
                if True:
                    xtile = xtp.tile([128, nxcf * 128], BF16, tag="xtile")
                    for fx in range(nxcf):
                        nc.sync.dma_start(
                            out=xtile[:, fx * 128:(fx + 1) * 128],
                            in_=xT[(rt * nxcf + fx) * 128:
                                   (rt * nxcf + fx + 1) * 128, :])
                    pm = pmmp.tile([128, max(hcols, 8)], F32, tag="pm")
                    pa = pamp.tile([128, 16], F32, tag="pa")
                    for fc in range(nfc):
                        if layer == 1:
                            lhsT = xtile[:, fc * 128: fc * 128 + vr]
                        elif fc < nxc:
                            lhsT = outT_sb[:, fc * NTILE + r0:
                                           fc * NTILE + r0 + vr]
                        else:
                            fx = fc - nxc
                            lhsT = xtile[:, fx * 128: fx * 128 + vr]
                        nc.tensor.matmul(out=pm[:vr, :hcols], lhsT=lhsT,
                                         rhs=Wm_sb[:, fc * hcols:(fc + 1) * hcols],
                                         start=(fc == 0), stop=(fc == nfc - 1))
                        nc.tensor.matmul(out=pa[:vr, :acols], lhsT=lhsT,
                                         rhs=Wa_sb[:, fc * acols:(fc + 1) * acols],
                                         start=(fc == 0), stop=(fc == nfc - 1))
                    zrow = mmzp.tile([128, trow], BF16, tag="zrow")
                    if rt < 4:
                        nc.vector.memset(zrow[:, hcols + acols:], 0.0)
                    nc.scalar.activation(
                        out=zrow[:vr, :hcols], in_=pm[:vr, :hcols],
                        func=mybir.ActivationFunctionType.Copy)
                    nc.scalar.activation(
                        out=zrow[:vr, hcols:hcols + 2 * nh].bitcast(F32),
                        in_=pa[:vr, 0:nh],
                        func=mybir.ActivationFunctionType.Copy)
                    nc.scalar.activation(
                        out=es_sb[layer][:vr, rt * nh:(rt + 1) * nh],
                        in_=pa[:vr, 0:nh],
                        func=mybir.ActivationFunctionType.Copy)
                    # ed -> resident SBUF bf16 table (scalar engine copy)
                    nc.scalar.activation(
                        out=ed_sb[layer][:vr, rt * nh:(rt + 1) * nh],
                        in_=pa[:vr, nh:2 * nh],
                        func=mybir.ActivationFunctionType.Copy)
                    tsh_w.append((rt, nc.sync.dma_start(
                        out=Tsh[r0:r0 + vr, :], in_=zrow[:vr, :])))

            # ---------- aggregation phase
            def agg_phase(layer, ccs, Tsh, tsh_w, post_pair=None):
                if layer == 3:
                    T_, trow, hcols, nh = T3, TROW3, OUT, 1
                    bias = bias_sb["b3"]
                else:
                    T_, trow, hcols, nh = (T1 if layer == 1 else T2), TROW, HC, HEADS
                    bias = bias_sb["b1"] if layer == 1 else bias_sb["b2"]
                esoff = hcols
                edt = ed_sb[layer]
                tshw_by_rt = dict(tsh_w)

                # batched self-loop weights: sx = exp(lrelu(es + ed))
                sx = sxp.tile([128, TPC * nh], BF16, tag="sx", name="sx")
                sxt = sxp.tile([128, TPC * nh], F32, tag="sxt", name="sxt")
                nc.vector.tensor_tensor(
                    out=sxt[:], in0=es_sb[layer][:, :TPC * nh],
                    in1=edt[:, :TPC * nh], op=mybir.AluOpType.add)
                sxt2 = sxp.tile([128, TPC * nh], F32, tag="sxt2", name="sxt2")
                nc.scalar.activation(
                    out=sxt2[:], in_=sxt[:],
                    func=mybir.ActivationFunctionType.Copy, scale=NEG_SLOPE)
                nc.vector.tensor_tensor(
                    out=sxt[:], in0=sxt[:], in1=sxt2[:],
                    op=mybir.AluOpType.max)
                nc.scalar.activation(
                    out=sx[:], in_=sxt[:],
                    func=mybir.ActivationFunctionType.Exp)

                for pair_i, pm_ in enumerate(pair_meta):
                    tiles = pm_["tiles"]
                    na_c, nb_c, nck = pm_["na_c"], pm_["nb_c"], pm_["nck"]
                    desc, m0 = pm_["desc"], pm_["m0"]
                    a0, b0 = pm_["a0"], pm_["b0"]
                    M = len(desc)
                    dodbg = DBG and layer == DBGL and pair_i == 0
                    dodbg_all = DBG and layer == DBGL

                    buf = gabp.tile([128, NCKMAX * trow], BF16, tag="buf")
                    gs = []
                    for cs in range(0, na_c, 8):
                        ck = min(8, na_c - cs)
                        gs.append(nc.gpsimd.dma_gather(
                            buf[:, cs * trow:(cs + ck) * trow].rearrange(
                                "p (c w) -> p c w", w=trow),
                            T_[:],
                            idxA_sb[:, (a0 + cs) * 8:(a0 + cs + ck) * 8],
                            ck * 128, numreg(ck * 128), trow))
                    for cs in range(0, nb_c, 8):
                        ck = min(8, nb_c - cs)
                        gs.append(nc.gpsimd.dma_gather(
                            buf[:, (na_c + cs) * trow:
                                (na_c + cs + ck) * trow].rearrange(
                                "p (c w) -> p c w", w=trow),
                            T_[ATH:],
                            idxB_sb[:, (b0 + cs) * 8:(b0 + cs + ck) * 8],
                            ck * 128, numreg(ck * 128), trow))
                    for g_ in gs:
                        for cc in ccs:
                            dep(g_, cc, "gather reads allgathered table")
                        for d_ in dummy_w[id(T_)]:
                            dep(g_, d_, "gather reads dummy rows")

                    # masks
                    mk = mkp.tile([128, MMAX * 128], BF16, tag="mk")
                    nc.sync.dma_start(
                        out=mk[:, :M * 128],
                        in_=msk_p[:, m0 * 128:(m0 + M) * 128])
                    mkT = mkTp.tile([128, MMAX * 128], BF16, tag="mkT")
                    nc.sync.dma_start(
                        out=mkT[:, :M * 128],
                        in_=mskT_p[:, m0 * 128:(m0 + M) * 128])

                    if dodbg:
                        nc.sync.dma_start(out=dbg["dbg_outT"][:, :],
                                          in_=outT_sb[:, :])
                        dT = nc.sync.dma_start(out=dbg["dbg_T1"][:, :trow],
                                               in_=T_[0:512, :])
                        for cc in ccs:
                            dep(dT, cc, "dbg reads table")
                        nc.sync.dma_start(out=dbg["dbg_buf"][:, :],
                                          in_=buf[:, :8 * trow])
                        nc.sync.dma_start(out=dbg["dbg_mk"][:, :],
                                          in_=mk[:, :8 * 128])
                        nc.sync.dma_start(out=dbg["dbg_mkT"][:, :],
                                          in_=mkT[:, :8 * 128])
                        nc.sync.dma_start(out=dbg["dbg_ed"][:, :],
                                          in_=ed_sb[1][:, :])

                    # eds: per chunk, sum over descriptors of mskT @ ed_tile
                    eds_ps = psEp.tile([128, max(NCKMAX * nh, 8)], F32,
                                       tag="eds")
                    by_chunk = {}
                    for m, (j, ti) in enumerate(desc):
                        by_chunk.setdefault(j, []).append((m, ti))
                    for j, ms in sorted(by_chunk.items()):
                        for q, (m, ti) in enumerate(ms):
                            tt = tiles[ti]
                            nc.tensor.matmul(
                                out=eds_ps[:, j * nh:(j + 1) * nh],
                                lhsT=mkT[:, m * 128:(m + 1) * 128],
                                rhs=edt[:, tt * nh:(tt + 1) * nh],
                                start=(q == 0), stop=(q == len(ms) - 1),
                                skip_group_check=True)

                    # logits: et = es + eds ; ex = exp(lrelu(et))
                    bv = buf[:].rearrange("p (c w) -> p c w", w=trow)
                    et = lgtp.tile([128, NCKMAX * nh], F32, tag="et")
                    nc.vector.tensor_tensor(
                        out=et[:, :nck * nh].rearrange(
                            "p (c h) -> p c h", h=nh),
                        in0=bv[:, 0:nck, esoff:esoff + 2 * nh].bitcast(F32),
                        in1=eds_ps[:, :nck * nh].rearrange(
                            "p (c h) -> p c h", h=nh),
                        op=mybir.AluOpType.add)
                    et2 = lgtp.tile([128, NCKMAX * nh], F32, tag="et2")
                    nc.scalar.activation(
                        out=et2[:, :nck * nh], in_=et[:, :nck * nh],
                        func=mybir.ActivationFunctionType.Copy,
                        scale=NEG_SLOPE)
                    nc.vector.tensor_tensor(
                        out=et2[:, :nck * nh], in0=et[:, :nck * nh],
                        in1=et2[:, :nck * nh], op=mybir.AluOpType.max)
                    ex = lgtp.tile([128, NCKMAX * nh], BF16, tag="ex")
                    nc.scalar.activation(
                        out=ex[:, :nck * nh], in_=et2[:, :nck * nh],
                        func=mybir.ActivationFunctionType.Exp)
                    if dodbg:
                        edscp = epp.tile([128, 32 * 8], F32, tag="edscp")
                        nc.vector.tensor_copy(out=edscp[:, :nck * nh],
                                              in_=eds_ps[:, :nck * nh])
                        nc.sync.dma_start(out=dbg["dbg_eds"][:, :],
                                          in_=edscp[:, :])
                        nc.sync.dma_start(out=dbg["dbg_et"][:, :nck * nh],
                                          in_=et[:, :nck * nh])
                        nc.sync.dma_start(out=dbg["dbg_ex"][:, :nck * nh],
                                          in_=ex[:, :nck * nh])

                    # scaled messages (in halves to bound SBUF)
                    pag = [psAp.tile([128, max(hcols, 8)], F32, tag="pag",
                                     name=f"pag{i}") for i in range(len(tiles))]
                    den = psDenp.tile([128, 16], F32, tag="den")
                    HALFMAX = (NCKMAX + 1) // 2
                    half = (nck + 1) // 2
                    mp_halves = []
                    for hi, h0 in enumerate(range(0, nck, half)):
                        h1 = min(h0 + half, nck)
                        mp_ = mpp.tile([128, HALFMAX * hcols], BF16,
                                       tag="mp", name=f"mp{hi}")
                        nc.vector.tensor_tensor(
                            out=mp_[:, :(h1 - h0) * hcols].rearrange(
                                "p (c h k) -> p c h k", h=nh, k=hcols // nh),
                            in0=bv[:, h0:h1, 0:hcols].rearrange(
                                "p c (h k) -> p c h k", h=nh),
                            in1=ex[:, h0 * nh:h1 * nh].rearrange(
                                "p (c h) -> p c h", h=nh)[:, :, :, None]
                                .to_broadcast(
                                    [128, h1 - h0, nh, hcols // nh]),
                            op=mybir.AluOpType.mult)
                        mp_halves.append((h0, h1, mp_))
                    # accumulation matmuls, tile-by-tile so each PSUM
                    # accumulation chain opens and closes sequentially
                    by_tile = {}
                    for m, (j, ti) in enumerate(desc):
                        by_tile.setdefault(ti, []).append((m, j))
                    for ti, ms in sorted(by_tile.items()):
                        for q, (m, j) in enumerate(ms):
                            st = (q == 0)
                            sp = (q == len(ms) - 1)
                            h0, h1, mp_ = next(
                                t for t in mp_halves
                                if t[0] <= j < t[1])
                            nc.tensor.matmul(
                                out=pag[ti][:, :hcols],
                                lhsT=mk[:, m * 128:(m + 1) * 128],
                                rhs=mp_[:, (j - h0) * hcols:
                                        (j - h0 + 1) * hcols],
                                start=st, stop=sp, skip_group_check=True)
                        for q, (m, j) in enumerate(ms):
                            nc.tensor.matmul(
                                out=den[:, ti * 8:ti * 8 + nh],
                                lhsT=mk[:, m * 128:(m + 1) * 128],
                                rhs=ex[:, j * nh:(j + 1) * nh],
                                start=(q == 0), stop=(q == len(ms) - 1),
                                skip_group_check=True)

                    # epilogue per tile (folds in the local self-loop term)
                    if dodbg_all:
                        dencp = epp.tile([128, 16], F32, tag="dencp")
                        nc.vector.tensor_copy(out=dencp[:], in_=den[:])
                        nc.sync.dma_start(
                            out=dbg["dbg_denall"][pair_i * 128:
                                                  (pair_i + 1) * 128, :],
                            in_=dencp[:, :])
                    for ti, tt in enumerate(tiles):
                        r0 = tt * 128
                        vr = min(128, R - r0)
                        if vr <= 0:
                            continue
                        zt = ztp.tile([128, max(hcols, 8)], BF16, tag="zt")
                        ztd = nc.sync.dma_start(out=zt[:vr, :hcols],
                                                in_=Tsh[r0:r0 + vr, 0:hcols])
                        dep(ztd, tshw_by_rt[tt], "self z reads shard write")
                        selfmp = epp.tile([128, max(hcols, 8)], F32,
                                          tag="selfmp")
                        nc.vector.tensor_tensor(
                            out=selfmp[:, :hcols].rearrange(
                                "p (h k) -> p h k", h=nh),
                            in0=zt[:, :hcols].rearrange(
                                "p (h k) -> p h k", h=nh),
                            in1=sx[:, tt * nh:(tt + 1) * nh, None]
                                .to_broadcast([128, nh, hcols // nh]),
                            op=mybir.AluOpType.mult)
                        num = epp.tile([128, max(hcols, 8)], F32, tag="num")
                        nc.vector.tensor_tensor(
                            out=num[:, :hcols], in0=pag[ti][:, :hcols],
                            in1=selfmp[:, :hcols], op=mybir.AluOpType.add)
                        dent = epp.tile([128, 8], F32, tag="dent")
                        nc.vector.tensor_tensor(
                            out=dent[:, :nh], in0=den[:, ti * 8:ti * 8 + nh],
                            in1=sx[:, tt * nh:(tt + 1) * nh],
                            op=mybir.AluOpType.add)
                        rden = epp.tile([128, 8], F32, tag="rden")
                        nc.vector.reciprocal(out=rden[:, :nh],
                                             in_=dent[:, :nh])
                        o1 = epp.tile([128, max(hcols, 8)], F32, tag="o1")
                        nc.vector.tensor_tensor(
                            out=o1[:, :hcols].rearrange(
                                "p (h k) -> p h k", h=nh),
                            in0=num[:, :hcols].rearrange(
                                "p (h k) -> p h k", h=nh),
                            in1=rden[:, :nh, None].to_broadcast(
                                [128, nh, hcols // nh]),
                            op=mybir.AluOpType.mult)
                        nc.vector.tensor_tensor(
                            out=o1[:, :hcols], in0=o1[:, :hcols],
                            in1=bias[:, :], op=mybir.AluOpType.add)
                        if dodbg_all and ti == 0:
                            nc.sync.dma_start(
                                out=dbg["dbg_o1all"][pair_i * 128:
                                                     pair_i * 128 + 128,
                                                     :hcols],
                                in_=o1[:, :hcols])
                        if layer != 3:
                            ob = epp.tile([128, hcols], BF16, tag="ob")
                            nc.scalar.activation(
                                out=ob[:, :], in_=o1[:, :hcols],
                                func=mybir.ActivationFunctionType.Relu)
                            pt = psTp.tile([128, (HC // 128) * 128], BF16,
                                           tag="pt")
                            for q in range(hcols // 128):
                                nc.tensor.transpose(
                                    out=pt[:, q * 128:q * 128 + vr],
                                    in_=ob[:vr, q * 128:(q + 1) * 128],
                                    identity=ident[:vr, :vr])
                            for q in range(hcols // 128):
                                nc.scalar.activation(
                                    out=outT_sb[:, q * NTILE + r0:
                                                q * NTILE + r0 + vr],
                                    in_=pt[:, q * 128:q * 128 + vr],
                                    func=mybir.ActivationFunctionType.Copy)
                        else:
                            mx = epp.tile([128, 1], F32, tag="mx")
                            nc.vector.tensor_reduce(
                                out=mx[:], in_=o1[:, :hcols],
                                op=mybir.AluOpType.max,
                                axis=mybir.AxisListType.X)
                            zc = epp.tile([128, hcols], F32, tag="zc")
                            nc.vector.tensor_scalar(
                                out=zc[:], in0=o1[:, :hcols], scalar1=mx[:],
                                scalar2=None,
                                op0=mybir.AluOpType.subtract)
                            ex3 = epp.tile([128, hcols], F32, tag="ex3")
                            s3 = epp.tile([128, 1], F32, tag="s3")
                            nc.scalar.activation(
                                out=ex3[:], in_=zc[:],
                                func=mybir.ActivationFunctionType.Exp,
                                accum_out=s3[:])
                            ln3 = epp.tile([128, 1], F32, tag="ln3")
                            nc.scalar.activation(
                                out=ln3[:], in_=s3[:],
                                func=mybir.ActivationFunctionType.Ln)
                            res = epp.tile([128, hcols], F32, tag="res")
                            nc.vector.tensor_scalar(
                                out=res[:], in0=zc[:], scalar1=ln3[:],
                                scalar2=None,
                                op0=mybir.AluOpType.subtract)
                            nc.sync.dma_start(out=out_ext[r0:r0 + vr, :],
                                              in_=res[:vr, :])
                    if post_pair is not None:
                        post_pair(pair_i, tiles)

            # ---------- the three layers, software-pipelined: layer l+1's
            # matmul tiles and AllGather chunks are emitted inside layer l's
            # aggregation pair loop so they execute during it.
            tables = {1: (T1sh, T1), 2: (T2sh, T2), 3: (T3sh, T3)}
            tshw = {1: [], 2: [], 3: []}
            ccsd = {1: [], 2: [], 3: []}

            def emit_ag(layer, k):
                Tsh_, T_ = tables[layer]
                t0, t1, off_k, rows_k = ag_chunks[k]
                r0 = t0 * 128
                cc = nc.gpsimd.collective_compute(
                    "AllGather",
                    mybir.AluOpType.bypass,
                    replica_groups=[list(range(P))],
                    ins=[Tsh_[r0:r0 + rows_k, :]],
                    outs=[T_[1 + off_k:1 + off_k + P * rows_k, :]],
                )
                for (rt, w_) in tshw[layer]:
                    if t0 <= rt < t1:
                        dep(cc, w_, "allgather reads shard chunk writes")
                ccsd[layer].append(cc)

            # fire AG chunks near the tail of the previous layer's
            # aggregation: early enough to mostly hide, late enough not to
            # contend with the bulk of the gather DMA traffic
            AG_FIRE_PAIR = cfg.get("AG_FIRE_PAIR", [17, 20, 23, 24])

            def make_post_pair(next_layer):
                def post_pair(pair_i, tiles):
                    for rt in tiles:
                        mm_tile(next_layer, rt, tshw[next_layer])
                    done = tiles[-1] + 1
                    for k, (t0, t1, off_k, rows_k) in enumerate(ag_chunks):
                        fire_at = max(AG_FIRE_PAIR[k],
                                      (t1 + 1) // 2 - 1 if t1 < TPC
                                      else len(pair_meta) - 1)
                        if pair_i == fire_at:
                            emit_ag(next_layer, k)
                return post_pair

            for rt in range(TPC):
                mm_tile(1, rt, tshw[1])
            for k in range(len(ag_chunks)):
                emit_ag(1, k)
            agg_phase(1, ccsd[1], T1sh, tshw[1], make_post_pair(2))
            agg_phase(2, ccsd[2], T2sh, tshw[2], make_post_pair(3))
            agg_phase(3, ccsd[3], T3sh, tshw[3])

    _regstack.close()
    from concourse.library_overlay import lower_extended_insts
    lower_extended_insts(nc)
    return nc


# --------------------------------------------------------------------------
# host wrapper
# --------------------------------------------------------------------------
def _prep_inputs(inputs, cfg):
    N, P, R, TPC = cfg["N"], cfg["P"], cfg["R"], cfg["TPC"]
    HEADS, HID, OUT, F_IN = cfg["HEADS"], cfg["HID"], cfg["OUT"], cfg["F_IN"]
    HC = HEADS * HID

    x = np.asarray(inputs["x"], np.float32)
    edge_index = np.asarray(inputs["edge_index"], np.int64)

    tile_ranges = cfg["TILE_RANGES"]
    perm, offs, rows_ks = build_perm(N, P, R, tile_ranges)
    ag_chunks = [(t0, t1, offs[k], rows_ks[k])
                 for k, (t0, t1) in enumerate(tile_ranges)]
    shards, pair_meta, CA, CB, Mtot = build_edges2(
        edge_index, N, P, R, TPC, cfg["ATH"], perm)

    def fold(W, a_s, a_d, heads, ch):
        F = W.shape[0]
        Wr = W.reshape(F, heads, ch)
        Wa = np.zeros((F, 2 * heads), np.float32)
        for h in range(heads):
            Wa[:, h] = Wr[:, h] @ a_s[h]
            Wa[:, heads + h] = Wr[:, h] @ a_d[h]
        return Wa

    w1 = np.asarray(inputs["w1"], np.float32)
    w2 = np.asarray(inputs["w2"], np.float32)
    w3 = np.asarray(inputs["w3"], np.float32)
    Wa1 = fold(w1, np.asarray(inputs["a1s"]), np.asarray(inputs["a1d"]),
               HEADS, HID)
    Wa2 = fold(w2, np.asarray(inputs["a2s"]), np.asarray(inputs["a2d"]),
               HEADS, HID)
    Wa3 = fold(w3, np.asarray(inputs["a3s"]), np.asarray(inputs["a3d"]),
               1, OUT)

    dum640 = np.zeros((1, cfg["TROW"]), BF16NP)
    dum640.view(np.uint8)[0, 2 * HC:2 * HC + HEADS * 4] = \
        np.full(HEADS, NEG_BIG, np.float32).view(np.uint8)
    dum128 = np.zeros((1, cfg["TROW3"]), BF16NP)
    dum128.view(np.uint8)[0, 2 * OUT:2 * OUT + 4] = \
        np.frombuffer(np.float32(NEG_BIG).tobytes(), np.uint8)

    common = {
        "Wm1": w1.astype(BF16NP), "Wa1": Wa1.astype(BF16NP),
        "Wm2": w2.astype(BF16NP), "Wa2": Wa2.astype(BF16NP),
        "Wm3": w3.astype(BF16NP), "Wa3": Wa3.astype(BF16NP),
        "b1": np.tile(np.asarray(inputs["b1"], np.float32).reshape(1, HC),
                      (128, 1)),
        "b2": np.tile(np.asarray(inputs["b2"], np.float32).reshape(1, HC),
                      (128, 1)),
        "b3": np.tile(np.asarray(inputs["b3"], np.float32).reshape(1, OUT),
                      (128, 1)),
        "dum640": dum640, "dum128": dum128,
    }
    TPC_ = cfg["TPC"]
    nxcf = F_IN // 128
    in_maps = []
    for p in range(P):
        m = dict(common)
        # tiled partition-contiguous layout: block (rt, fc) = [128 feat,
        # 128 nodes] contiguous, so each tile load is one linear 32KB read
        xp = np.zeros((TPC_ * 128, F_IN), np.float32)
        xp[:R] = x[p * R:(p + 1) * R, :]
        xt4 = xp.reshape(TPC_, 128, nxcf, 128).transpose(0, 2, 3, 1)
        m["xT"] = np.ascontiguousarray(
            xt4.reshape(TPC_ * nxcf * 128, 128)).astype(BF16NP)
        m["idxA"] = shards[p]["idxA"]
        m["idxB"] = shards[p]["idxB"]
        m["msk"] = shards[p]["msk"]
        m["mskT"] = shards[p]["mskT"]
        in_maps.append(m)
    return in_maps, pair_meta, CA, CB, Mtot, ag_chunks


def default_cfg():
    return dict(N=50000, P=8, R=6250, TPC=49, F_IN=256, HID=64, HEADS=8,
                OUT=16, ATH=32768, TROW=640, TROW3=128,
                NEG_SLOPE=0.2,
                TILE_RANGES=[(0, 14), (14, 28), (28, 44), (44, 49)])


def kernel(**inputs):
    cfg = default_cfg()
    in_maps, pair_meta, CA, CB, Mtot, ag_chunks = _prep_inputs(inputs, cfg)
    nc = build_program(cfg, pair_meta, CA, CB, Mtot, ag_chunks)

    _split_multiwait(nc)
    from concourse.bass_utils import run_bass_kernel_spmd
    trace = bool(os.environ.get("GNN_TRACE"))
    if trace:
        sys.path.insert(0, "/root/problem/work")
        import axonhook  # noqa
    res = run_bass_kernel_spmd(nc, in_maps, list(range(cfg["P"])),
                               trace=trace)
    if trace:
        kernel.last_exec_ns = res.exec_time_ns
    if os.environ.get("GNN_DEBUG"):
        np.savez("/root/problem/work/dbg.npz",
                 **{k: np.asarray(v) for k, v in res.results[0].items()
                    if k.startswith("dbg_")})
    out = np.concatenate([res.results[p]["out"] for p in range(cfg["P"])],
                         axis=0)
    return out.astype(np.float32)
